# revision 2
# baseline (speedup 1.0000x reference)
"""LocalAggregationLoss on 8 TRN2 NeuronCores (Bass/Tile).

loss = mean_b( log(sum_n mask_bg*exp(v@bank.T/T)) - log(sum_n mask_int*exp(...)) )

Strategy: shard bank+masks along N across 8 cores. Per core:
  - normalize codes on-device, PE-transpose -> vT/8 (f32r)
  - dots/8 via f32r matmul; += 0.875*mask_bg via fp8e5-subnormal-bitcast
    identity matmul into the same PSUM accumulation group
  - ACT Exp(scale*x + bias) with fused per-row accumulate -> d1 partials
    (masked-out elements become exp(dots/T - 100) == 0)
  - DVE scalar_tensor_tensor(e * mask_int) with fused accumulate -> d2
    partials (mask_int subset of mask_bg, so e is already bg-masked)
  - AllReduce [128,4] partial sums, then log/sub/sum -> scalar loss
"""

import os
import sys

for _p in ("/opt/trn_rl_repo", "/root/.axon_site/_ro/trn_rl_repo"):
    if os.path.isdir(_p) and _p not in sys.path:
        sys.path.insert(0, _p)

import numpy as np
import concourse.bacc as bacc
import concourse.tile as tile
from concourse import mybir
from concourse.bass_utils import run_bass_kernel_spmd
from concourse.masks import make_identity

dt = mybir.dt

# problem constants (hardcoded per contract)
B, N, D = 256, 200000, 128
TEMP = 0.07
NCORES = 8
S = N // NCORES  # 25000 per-core shard
CHUNK = 512
CHUNKS_PER_GROUP = 7
NGROUPS = 7
NCHUNK = CHUNKS_PER_GROUP * NGROUPS  # 49
S_PAD = NCHUNK * CHUNK  # 25088
GROUP = CHUNKS_PER_GROUP * CHUNK  # 3584

ALPHA = 1.0 / 8.0  # dots prescale (folded into vT)
C_FP8 = 0.875  # mask coefficient in psum (= 57344 * 2^-16)
FP8_SCALE = float(C_FP8 * 2**16)  # 57344 == max normal fp8e5
ACT_SCALE = 1.0 / (ALPHA * TEMP)  # 114.2857...
ACT_BIAS = -C_FP8 / (ALPHA * TEMP)  # -100.0

_CACHE = {}


def _build():
    nc = bacc.Bacc("TRN2", target_bir_lowering=False, debug=False, num_devices=NCORES)
    codes_d = nc.dram_tensor("codes", [B, D], dt.float32, kind="ExternalInput").ap()
    bankT_d = nc.dram_tensor("bankT", [D, S_PAD], dt.float32r, kind="ExternalInput").ap()
    mbg_d = nc.dram_tensor("mbg", [B, S_PAD], dt.uint8, kind="ExternalInput").ap()
    mint_d = nc.dram_tensor("mint", [B, S_PAD], dt.uint8, kind="ExternalInput").ap()
    out_d = nc.dram_tensor("out", [1, 1], dt.float32, kind="ExternalOutput").ap()

    with tile.TileContext(nc) as tc:
        with (
            tc.tile_pool(name="const", bufs=1) as constp,
            tc.tile_pool(name="vprep", bufs=1) as vprep,
            tc.tile_pool(name="bank", bufs=2) as bankp,
            tc.tile_pool(name="mask", bufs=2) as maskp,
            tc.tile_pool(name="ework", bufs=4) as ework,
            tc.tile_pool(name="scratch", bufs=2) as scratch,
            tc.tile_pool(name="ps", bufs=4, space="PSUM") as ps,
            tc.tile_pool(name="psv", bufs=1, space="PSUM") as psv,
            tc.tile_pool(name="dram", bufs=1, space="DRAM") as dram,
        ):
            # ---- constants ----
            ident_f32 = constp.tile([128, 128], dt.float32)
            make_identity(nc, ident_f32[:])
            ident_fp8 = constp.tile([128, 128], dt.float8e5)
            nc.vector.tensor_scalar(
                out=ident_fp8[:],
                in0=ident_f32[:],
                scalar1=FP8_SCALE,
                scalar2=None,
                op0=mybir.AluOpType.mult,
            )
            bias_t = constp.tile([128, 1], dt.float32)
            nc.gpsimd.memset(bias_t[:], ACT_BIAS)
            ones_t = constp.tile([128, 1], dt.float32)
            nc.gpsimd.memset(ones_t[:], 1.0)

            # ---- phase A: normalize codes, build vT/8 (f32r) ----
            vT = []
            for h in range(2):
                codes_t = vprep.tile([128, D], dt.float32, tag=f"codes{h}")
                nc.sync.dma_start(out=codes_t[:], in_=codes_d[h * 128 : (h + 1) * 128, :])
                sq_t = vprep.tile([128, D], dt.float32, tag=f"sq{h}")
                ss_t = vprep.tile([128, 1], dt.float32, tag=f"ss{h}")
                nc.scalar.activation(
                    out=sq_t[:],
                    in_=codes_t[:],
                    func=mybir.ActivationFunctionType.Square,
                    accum_out=ss_t[:],
                )
                # 8*norm = sqrt(64*ss)
                n8_t = vprep.tile([128, 1], dt.float32, tag=f"n8{h}")
                nc.scalar.activation(
                    out=n8_t[:],
                    in_=ss_t[:],
                    func=mybir.ActivationFunctionType.Sqrt,
                    scale=64.0,
                )
                rn_t = vprep.tile([128, 1], dt.float32, tag=f"rn{h}")
                nc.vector.reciprocal(out=rn_t[:], in_=n8_t[:])
                v_t = vprep.tile([128, D], dt.float32, tag=f"v{h}")
                nc.scalar.activation(
                    out=v_t[:],
                    in_=codes_t[:],
                    func=mybir.ActivationFunctionType.Copy,
                    scale=rn_t[:],
                )
                psv_t = psv.tile([128, 128], dt.float32, tag=f"psv{h}")
                nc.tensor.transpose(out=psv_t[:], in_=v_t[:], identity=ident_f32[:])
                vT_t = vprep.tile([128, 128], dt.float32r, tag=f"vT{h}")
                nc.vector.tensor_copy(out=vT_t[:], in_=psv_t[:])
                vT.append(vT_t)

            # ---- phase B: main loop ----
            d1_strip = [constp.tile([128, NCHUNK], dt.float32, name=f"d1s{h}", tag=f"d1s{h}") for h in range(2)]
            d2_strip = [constp.tile([128, NCHUNK], dt.float32, name=f"d2s{h}", tag=f"d2s{h}") for h in range(2)]

            for g in range(NGROUPS):
                g0 = g * GROUP
                bank_g = bankp.tile([128, GROUP], dt.float32r, tag="bank")
                nc.sync.dma_start(out=bank_g[:], in_=bankT_d[:, g0 : g0 + GROUP])
                m_tiles = {}
                for h in range(2):
                    r0 = h * 128
                    mbg_g = maskp.tile([128, GROUP], dt.uint8, tag=f"mbg{h}")
                    nc.sync.dma_start(
                        out=mbg_g[:], in_=mbg_d[r0 : r0 + 128, g0 : g0 + GROUP]
                    )
                    mint_g = maskp.tile([128, GROUP], dt.uint8, tag=f"mint{h}")
                    nc.sync.dma_start(
                        out=mint_g[:], in_=mint_d[r0 : r0 + 128, g0 : g0 + GROUP]
                    )
                    m_tiles[h] = (mbg_g, mint_g)

                for h in range(2):
                    mbg_g, mint_g = m_tiles[h]
                    for ci in range(CHUNKS_PER_GROUP):
                        c0 = ci * CHUNK
                        kidx = g * CHUNKS_PER_GROUP + ci
                        psum_t = ps.tile([128, CHUNK], dt.float32, tag="psum")
                        nc.tensor.matmul(
                            out=psum_t[:],
                            lhsT=vT[h][:],
                            rhs=bank_g[:, c0 : c0 + CHUNK],
                            start=True,
                            stop=False,
                        )
                        nc.tensor.matmul(
                            out=psum_t[:],
                            lhsT=ident_fp8[:],
                            rhs=mbg_g[:, c0 : c0 + CHUNK].bitcast(dt.float8e5),
                            start=False,
                            stop=True,
                        )
                        e_t = ework.tile([128, CHUNK], dt.float32, tag="e")
                        nc.scalar.activation(
                            out=e_t[:],
                            in_=psum_t[:],
                            func=mybir.ActivationFunctionType.Exp,
                            scale=ACT_SCALE,
                            bias=bias_t[:],
                            accum_out=d1_strip[h][:, kidx : kidx + 1],
                        )
                        stt_s = scratch.tile([128, CHUNK], dt.float32, tag="stt")
                        nc.vector.scalar_tensor_tensor(
                            out=stt_s[:],
                            in0=e_t[:],
                            scalar=0.0,
                            in1=mint_g[:, c0 : c0 + CHUNK],
                            op0=mybir.AluOpType.add,
                            op1=mybir.AluOpType.mult,
                            accum_out=d2_strip[h][:, kidx : kidx + 1],
                        )

            # ---- phase C: finale ----
            # partials [128, 4]: cols = d1_h0, d1_h1, d2_h0, d2_h1
            parts_t = constp.tile([128, 4], dt.float32)
            for h in range(2):
                nc.vector.tensor_reduce(
                    out=parts_t[:, h : h + 1],
                    in_=d1_strip[h][:],
                    axis=mybir.AxisListType.X,
                    op=mybir.AluOpType.add,
                )
                nc.vector.tensor_reduce(
                    out=parts_t[:, 2 + h : 3 + h],
                    in_=d2_strip[h][:],
                    axis=mybir.AxisListType.X,
                    op=mybir.AluOpType.add,
                )

            cc_in = dram.tile([128, 4], dt.float32)
            cc_out = dram.tile([128, 4], dt.float32)
            nc.sync.dma_start(out=cc_in[:], in_=parts_t[:])
            nc.gpsimd.collective_compute(
                "AllReduce",
                mybir.AluOpType.add,
                replica_groups=[list(range(NCORES))],
                ins=[cc_in.opt()],
                outs=[cc_out.opt()],
            )
            sums_t = constp.tile([128, 4], dt.float32)
            nc.sync.dma_start(out=sums_t[:], in_=cc_out[:])

            ln_t = constp.tile([128, 4], dt.float32)
            nc.scalar.activation(
                out=ln_t[:], in_=sums_t[:], func=mybir.ActivationFunctionType.Ln
            )
            ldiff_t = constp.tile([128, 2], dt.float32)
            nc.vector.tensor_sub(out=ldiff_t[:], in0=ln_t[:, 0:2], in1=ln_t[:, 2:4])
            lsum_t = constp.tile([128, 1], dt.float32)
            nc.vector.tensor_reduce(
                out=lsum_t[:],
                in_=ldiff_t[:],
                axis=mybir.AxisListType.X,
                op=mybir.AluOpType.add,
            )
            # partition sum via ones-matmul: out[1,1] = sum_k lsum[k]*1
            psum_s = psv.tile([1, 1], dt.float32, tag="psum_s")
            nc.tensor.matmul(
                out=psum_s[:], lhsT=lsum_t[:], rhs=ones_t[:], start=True, stop=True
            )
            out_t = constp.tile([1, 1], dt.float32)
            nc.scalar.activation(
                out=out_t[:],
                in_=psum_s[:],
                func=mybir.ActivationFunctionType.Copy,
                scale=1.0 / B,
            )
            nc.sync.dma_start(out=out_d[:], in_=out_t[:])

    nc.compile()
    return nc


def _get_nc():
    if "nc" not in _CACHE:
        _CACHE["nc"] = _build()
    return _CACHE["nc"]


def make_in_maps(codes, bank, mask_bg, mask_int):
    codes = np.ascontiguousarray(np.asarray(codes, dtype=np.float32))
    bank = np.asarray(bank, dtype=np.float32)
    mbg_u8 = np.asarray(mask_bg).view(np.uint8) if np.asarray(mask_bg).dtype == np.bool_ else np.asarray(mask_bg).astype(np.uint8)
    mint_u8 = np.asarray(mask_int).view(np.uint8) if np.asarray(mask_int).dtype == np.bool_ else np.asarray(mask_int).astype(np.uint8)

    in_maps = []
    for c in range(NCORES):
        n0, n1 = c * S, (c + 1) * S
        bankT_c = np.zeros((D, S_PAD), dtype=np.float32)
        bankT_c[:, :S] = bank[n0:n1].T
        mbg_c = np.zeros((B, S_PAD), dtype=np.uint8)
        mbg_c[:, :S] = mbg_u8[:, n0:n1]
        mint_c = np.zeros((B, S_PAD), dtype=np.uint8)
        mint_c[:, :S] = mint_u8[:, n0:n1]
        in_maps.append(
            {
                "codes": codes,
                "bankT": np.ascontiguousarray(bankT_c),
                "mbg": mbg_c,
                "mint": mint_c,
            }
        )
    return in_maps


def kernel(codes, bank, mask_bg, mask_int):
    nc = _get_nc()
    in_maps = make_in_maps(codes, bank, mask_bg, mask_int)
    res = run_bass_kernel_spmd(nc, in_maps, core_ids=list(range(NCORES)))
    return np.float32(res.results[0]["out"][0, 0])


# revision 3
# speedup vs baseline: 8.3621x; 8.3621x over previous
"""LocalAggregationLoss on 8 TRN2 NeuronCores (Bass/Tile).

loss = mean_b( log(sum_n mask_bg*exp(v@bank.T/T)) - log(sum_n mask_int*exp(...)) )

Strategy: shard bank+masks along N across 8 cores. Per core:
  - normalize codes on-device, PE-transpose -> vT/8 (f32r)
  - dots/8 via f32r matmul; += 0.875*mask_bg via fp8e5-subnormal-bitcast
    identity matmul into the same PSUM accumulation group
  - ACT Exp(scale*x + bias) with fused per-row accumulate -> d1 partials
    (masked-out elements become exp(dots/T - 100) == 0)
  - DVE scalar_tensor_tensor(e * mask_int) with fused accumulate -> d2
    partials (mask_int subset of mask_bg, so e is already bg-masked)
  - AllReduce [128,4] partial sums, then log/sub/sum -> scalar loss
"""

import os
import sys

for _p in ("/opt/trn_rl_repo", "/root/.axon_site/_ro/trn_rl_repo"):
    if os.path.isdir(_p) and _p not in sys.path:
        sys.path.insert(0, _p)

import numpy as np
import concourse.bacc as bacc
import concourse.tile as tile
from concourse import mybir
from concourse.bass_utils import run_bass_kernel_spmd
from concourse.masks import make_identity

dt = mybir.dt

# problem constants (hardcoded per contract)
B, N, D = 256, 200000, 128
TEMP = 0.07
NCORES = 8
S = N // NCORES  # 25000 per-core shard
CHUNK = 512
CHUNKS_PER_GROUP = 7
NGROUPS = 7
NCHUNK = CHUNKS_PER_GROUP * NGROUPS  # 49
S_PAD = NCHUNK * CHUNK  # 25088
GROUP = CHUNKS_PER_GROUP * CHUNK  # 3584

ALPHA = 1.0 / 8.0  # dots prescale (folded into vT)
C_FP8 = 0.875  # mask coefficient in psum (= 57344 * 2^-16)
FP8_SCALE = float(C_FP8 * 2**16)  # 57344 == max normal fp8e5
ACT_SCALE = 1.0 / (ALPHA * TEMP)  # 114.2857...
ACT_BIAS = -C_FP8 / (ALPHA * TEMP)  # -100.0

_CACHE = {}


def _build(reps: int = 1):
    nc = bacc.Bacc("TRN2", target_bir_lowering=False, debug=False, num_devices=NCORES)
    codes_d = nc.dram_tensor("codes", [B, D], dt.float32, kind="ExternalInput").ap()
    bankT_d = nc.dram_tensor("bankT", [D, S_PAD], dt.float32r, kind="ExternalInput").ap()
    mbg_d = nc.dram_tensor("mbg", [B, S_PAD], dt.uint8, kind="ExternalInput").ap()
    mint_d = nc.dram_tensor("mint", [B, S_PAD], dt.uint8, kind="ExternalInput").ap()
    out_d = nc.dram_tensor("out", [1, 1], dt.float32, kind="ExternalOutput").ap()

    with tile.TileContext(nc) as tc:
        with (
            tc.tile_pool(name="const", bufs=1) as constp,
            tc.tile_pool(name="vprep", bufs=1) as vprep,
            tc.tile_pool(name="bank", bufs=2) as bankp,
            tc.tile_pool(name="mask", bufs=2) as maskp,
            tc.tile_pool(name="ework", bufs=4) as ework,
            tc.tile_pool(name="scratch", bufs=2) as scratch,
            tc.tile_pool(name="ps", bufs=4, space="PSUM") as ps,
            tc.tile_pool(name="psv", bufs=1, space="PSUM") as psv,
            tc.tile_pool(name="dram", bufs=1, space="DRAM") as dram,
        ):
            # ---- constants ----
            ident_f32 = constp.tile([128, 128], dt.float32)
            make_identity(nc, ident_f32[:])
            ident_fp8 = constp.tile([128, 128], dt.float8e5)
            nc.vector.tensor_scalar(
                out=ident_fp8[:],
                in0=ident_f32[:],
                scalar1=FP8_SCALE,
                scalar2=None,
                op0=mybir.AluOpType.mult,
            )
            bias_t = constp.tile([128, 1], dt.float32)
            nc.gpsimd.memset(bias_t[:], ACT_BIAS)
            ones_t = constp.tile([128, 1], dt.float32)
            nc.gpsimd.memset(ones_t[:], 1.0)

            # ---- phase A: normalize codes, build vT/8 (f32r) ----
            vT = []
            for h in range(2):
                codes_t = vprep.tile([128, D], dt.float32, tag=f"codes{h}")
                nc.sync.dma_start(out=codes_t[:], in_=codes_d[h * 128 : (h + 1) * 128, :])
                sq_t = vprep.tile([128, D], dt.float32, tag=f"sq{h}")
                ss_t = vprep.tile([128, 1], dt.float32, tag=f"ss{h}")
                nc.scalar.activation(
                    out=sq_t[:],
                    in_=codes_t[:],
                    func=mybir.ActivationFunctionType.Square,
                    accum_out=ss_t[:],
                )
                # 8*norm = sqrt(64*ss)
                n8_t = vprep.tile([128, 1], dt.float32, tag=f"n8{h}")
                nc.scalar.activation(
                    out=n8_t[:],
                    in_=ss_t[:],
                    func=mybir.ActivationFunctionType.Sqrt,
                    scale=64.0,
                )
                rn_t = vprep.tile([128, 1], dt.float32, tag=f"rn{h}")
                nc.vector.reciprocal(out=rn_t[:], in_=n8_t[:])
                v_t = vprep.tile([128, D], dt.float32, tag=f"v{h}")
                nc.scalar.activation(
                    out=v_t[:],
                    in_=codes_t[:],
                    func=mybir.ActivationFunctionType.Copy,
                    scale=rn_t[:],
                )
                psv_t = psv.tile([128, 128], dt.float32, tag=f"psv{h}")
                nc.tensor.transpose(out=psv_t[:], in_=v_t[:], identity=ident_f32[:])
                vT_t = vprep.tile([128, 128], dt.float32r, tag=f"vT{h}")
                nc.vector.tensor_copy(out=vT_t[:], in_=psv_t[:])
                vT.append(vT_t)

            # ---- phase B: main loop ----
            d1_strip = [constp.tile([128, NCHUNK], dt.float32, name=f"d1s{h}", tag=f"d1s{h}") for h in range(2)]
            d2_strip = [constp.tile([128, NCHUNK], dt.float32, name=f"d2s{h}", tag=f"d2s{h}") for h in range(2)]

            import contextlib

            loop_cm = tc.For_i(0, reps, 1) if reps > 1 else contextlib.nullcontext()
            with loop_cm:
              for g in range(NGROUPS):
                g0 = g * GROUP
                bank_g = bankp.tile([128, GROUP], dt.float32r, tag="bank")
                nc.sync.dma_start(out=bank_g[:], in_=bankT_d[:, g0 : g0 + GROUP])
                m_tiles = {}
                for h in range(2):
                    r0 = h * 128
                    mbg_g = maskp.tile([128, GROUP], dt.uint8, tag=f"mbg{h}")
                    nc.sync.dma_start(
                        out=mbg_g[:], in_=mbg_d[r0 : r0 + 128, g0 : g0 + GROUP]
                    )
                    mint_g = maskp.tile([128, GROUP], dt.uint8, tag=f"mint{h}")
                    nc.sync.dma_start(
                        out=mint_g[:], in_=mint_d[r0 : r0 + 128, g0 : g0 + GROUP]
                    )
                    m_tiles[h] = (mbg_g, mint_g)

                for h in range(2):
                    mbg_g, mint_g = m_tiles[h]
                    for ci in range(CHUNKS_PER_GROUP):
                        c0 = ci * CHUNK
                        kidx = g * CHUNKS_PER_GROUP + ci
                        psum_t = ps.tile([128, CHUNK], dt.float32, tag="psum")
                        nc.tensor.matmul(
                            out=psum_t[:],
                            lhsT=vT[h][:],
                            rhs=bank_g[:, c0 : c0 + CHUNK],
                            start=True,
                            stop=False,
                        )
                        nc.tensor.matmul(
                            out=psum_t[:],
                            lhsT=ident_fp8[:],
                            rhs=mbg_g[:, c0 : c0 + CHUNK].bitcast(dt.float8e5),
                            start=False,
                            stop=True,
                        )
                        e_t = ework.tile([128, CHUNK], dt.float32, tag="e")
                        nc.scalar.activation(
                            out=e_t[:],
                            in_=psum_t[:],
                            func=mybir.ActivationFunctionType.Exp,
                            scale=ACT_SCALE,
                            bias=bias_t[:],
                            accum_out=d1_strip[h][:, kidx : kidx + 1],
                        )
                        stt_s = scratch.tile([128, CHUNK], dt.float32, tag="stt")
                        nc.vector.scalar_tensor_tensor(
                            out=stt_s[:],
                            in0=e_t[:],
                            scalar=0.0,
                            in1=mint_g[:, c0 : c0 + CHUNK],
                            op0=mybir.AluOpType.add,
                            op1=mybir.AluOpType.mult,
                            accum_out=d2_strip[h][:, kidx : kidx + 1],
                        )

            # ---- phase C: finale ----
            # partials [128, 4]: cols = d1_h0, d1_h1, d2_h0, d2_h1
            parts_t = constp.tile([128, 4], dt.float32)
            for h in range(2):
                nc.vector.tensor_reduce(
                    out=parts_t[:, h : h + 1],
                    in_=d1_strip[h][:],
                    axis=mybir.AxisListType.X,
                    op=mybir.AluOpType.add,
                )
                nc.vector.tensor_reduce(
                    out=parts_t[:, 2 + h : 3 + h],
                    in_=d2_strip[h][:],
                    axis=mybir.AxisListType.X,
                    op=mybir.AluOpType.add,
                )

            cc_in = dram.tile([128, 4], dt.float32)
            cc_out = dram.tile([128, 4], dt.float32)
            nc.sync.dma_start(out=cc_in[:], in_=parts_t[:])
            nc.gpsimd.collective_compute(
                "AllReduce",
                mybir.AluOpType.add,
                replica_groups=[list(range(NCORES))],
                ins=[cc_in.opt()],
                outs=[cc_out.opt()],
            )
            sums_t = constp.tile([128, 4], dt.float32)
            nc.sync.dma_start(out=sums_t[:], in_=cc_out[:])

            ln_t = constp.tile([128, 4], dt.float32)
            nc.scalar.activation(
                out=ln_t[:], in_=sums_t[:], func=mybir.ActivationFunctionType.Ln
            )
            ldiff_t = constp.tile([128, 2], dt.float32)
            nc.vector.tensor_sub(out=ldiff_t[:], in0=ln_t[:, 0:2], in1=ln_t[:, 2:4])
            lsum_t = constp.tile([128, 1], dt.float32)
            nc.vector.tensor_reduce(
                out=lsum_t[:],
                in_=ldiff_t[:],
                axis=mybir.AxisListType.X,
                op=mybir.AluOpType.add,
            )
            # partition sum via ones-matmul: out[1,1] = sum_k lsum[k]*1
            psum_s = psv.tile([1, 1], dt.float32, tag="psum_s")
            nc.tensor.matmul(
                out=psum_s[:], lhsT=lsum_t[:], rhs=ones_t[:], start=True, stop=True
            )
            out_t = constp.tile([1, 1], dt.float32)
            nc.scalar.activation(
                out=out_t[:],
                in_=psum_s[:],
                func=mybir.ActivationFunctionType.Copy,
                scale=1.0 / B,
            )
            nc.sync.dma_start(out=out_d[:], in_=out_t[:])

    nc.compile()
    return nc


def _get_nc(reps: int = 1):
    key = ("nc", reps)
    if key not in _CACHE:
        _CACHE[key] = _build(reps)
    return _CACHE[key]


def make_in_maps(codes, bank, mask_bg, mask_int):
    codes = np.ascontiguousarray(np.asarray(codes, dtype=np.float32))
    bank = np.asarray(bank, dtype=np.float32)
    mbg_u8 = np.asarray(mask_bg).view(np.uint8) if np.asarray(mask_bg).dtype == np.bool_ else np.asarray(mask_bg).astype(np.uint8)
    mint_u8 = np.asarray(mask_int).view(np.uint8) if np.asarray(mask_int).dtype == np.bool_ else np.asarray(mask_int).astype(np.uint8)

    in_maps = []
    for c in range(NCORES):
        n0, n1 = c * S, (c + 1) * S
        bankT_c = np.zeros((D, S_PAD), dtype=np.float32)
        bankT_c[:, :S] = bank[n0:n1].T
        mbg_c = np.zeros((B, S_PAD), dtype=np.uint8)
        mbg_c[:, :S] = mbg_u8[:, n0:n1]
        mint_c = np.zeros((B, S_PAD), dtype=np.uint8)
        mint_c[:, :S] = mint_u8[:, n0:n1]
        in_maps.append(
            {
                "codes": codes,
                "bankT": np.ascontiguousarray(bankT_c),
                "mbg": mbg_c,
                "mint": mint_c,
            }
        )
    return in_maps


def kernel(codes, bank, mask_bg, mask_int):
    nc = _get_nc()
    in_maps = make_in_maps(codes, bank, mask_bg, mask_int)
    res = run_bass_kernel_spmd(nc, in_maps, core_ids=list(range(NCORES)))
    return np.float32(res.results[0]["out"][0, 0])


# revision 6
# speedup vs baseline: 8.7287x; 1.0438x over previous
"""LocalAggregationLoss on 8 TRN2 NeuronCores (Bass/Tile).

loss = mean_b( log(sum_n mask_bg*exp(v@bank.T/T)) - log(sum_n mask_int*exp(...)) )

Strategy: shard bank+masks along N across 8 cores. Per core:
  - normalize codes on-device, PE-transpose -> vT/8 (f32r)
  - dots/8 via f32r matmul; += 0.875*mask_bg via fp8e5-subnormal-bitcast
    identity matmul into the same PSUM accumulation group
  - ACT Exp(scale*x + bias) with fused per-row accumulate -> d1 partials
    (masked-out elements become exp(dots/T - 100) == 0)
  - DVE scalar_tensor_tensor(e * mask_int) with fused accumulate -> d2
    partials (mask_int subset of mask_bg, so e is already bg-masked)
  - AllReduce [128,4] partial sums, then log/sub/sum -> scalar loss
"""

import os
import sys

for _p in ("/opt/trn_rl_repo", "/root/.axon_site/_ro/trn_rl_repo"):
    if os.path.isdir(_p) and _p not in sys.path:
        sys.path.insert(0, _p)

import numpy as np
import concourse.bacc as bacc
import concourse.tile as tile
from concourse import mybir
from concourse.bass_utils import run_bass_kernel_spmd
from concourse.masks import make_identity

dt = mybir.dt

# problem constants (hardcoded per contract)
B, N, D = 256, 200000, 128
TEMP = 0.07
NCORES = 8
S = N // NCORES  # 25000 per-core shard
CHUNK = 512
CHUNKS_PER_GROUP = 7
NGROUPS = 7
NCHUNK = CHUNKS_PER_GROUP * NGROUPS  # 49
S_PAD = NCHUNK * CHUNK  # 25088
GROUP = CHUNKS_PER_GROUP * CHUNK  # 3584

ALPHA = 1.0 / 8.0  # dots prescale (folded into vT)
C_FP8 = 0.875  # mask coefficient in psum (= 57344 * 2^-16)
FP8_SCALE = float(C_FP8 * 2**16)  # 57344 == max normal fp8e5
ACT_SCALE = 1.0 / (ALPHA * TEMP)  # 114.2857...
ACT_BIAS = -C_FP8 / (ALPHA * TEMP)  # -100.0

_CACHE = {}


def _build(reps: int = 1):
    nc = bacc.Bacc("TRN2", target_bir_lowering=False, debug=False, num_devices=NCORES)
    codes_d = nc.dram_tensor("codes", [B, D], dt.float32, kind="ExternalInput").ap()
    bankT_d = nc.dram_tensor("bankT", [D, S_PAD], dt.float32r, kind="ExternalInput").ap()
    mbg_d = nc.dram_tensor("mbg", [B, S_PAD], dt.uint8, kind="ExternalInput").ap()
    mint_d = nc.dram_tensor("mint", [B, S_PAD], dt.uint8, kind="ExternalInput").ap()
    out_d = nc.dram_tensor("out", [1, 1], dt.float32, kind="ExternalOutput").ap()

    with tile.TileContext(nc) as tc:
        with (
            tc.tile_pool(name="const", bufs=1) as constp,
            tc.tile_pool(name="vprep", bufs=1) as vprep,
            tc.tile_pool(name="bank", bufs=3) as bankp,
            tc.tile_pool(name="mask", bufs=3) as maskp,
            tc.tile_pool(name="ework", bufs=3) as ework,
            tc.tile_pool(name="scratch", bufs=2) as scratch,
            tc.tile_pool(name="ps", bufs=3, space="PSUM") as ps,
            tc.tile_pool(name="psv", bufs=1, space="PSUM") as psv,
            tc.tile_pool(name="dram", bufs=1, space="DRAM") as dram,
        ):
            # ---- constants ----
            ident_f32 = constp.tile([128, 128], dt.float32)
            make_identity(nc, ident_f32[:])
            ident_fp8 = constp.tile([128, 128], dt.float8e5)
            nc.vector.tensor_scalar(
                out=ident_fp8[:],
                in0=ident_f32[:],
                scalar1=FP8_SCALE,
                scalar2=None,
                op0=mybir.AluOpType.mult,
            )
            bias_t = constp.tile([128, 1], dt.float32)
            nc.gpsimd.memset(bias_t[:], ACT_BIAS)
            ones_t = constp.tile([128, 1], dt.float32)
            nc.gpsimd.memset(ones_t[:], 1.0)

            # ---- phase A: normalize codes, build vT/8 (f32r) ----
            vT = []
            for h in range(2):
                codes_t = vprep.tile([128, D], dt.float32, tag=f"codes{h}")
                nc.sync.dma_start(out=codes_t[:], in_=codes_d[h * 128 : (h + 1) * 128, :])
                sq_t = vprep.tile([128, D], dt.float32, tag=f"sq{h}")
                ss_t = vprep.tile([128, 1], dt.float32, tag=f"ss{h}")
                nc.scalar.activation(
                    out=sq_t[:],
                    in_=codes_t[:],
                    func=mybir.ActivationFunctionType.Square,
                    accum_out=ss_t[:],
                )
                # 8*norm = sqrt(64*ss)
                n8_t = vprep.tile([128, 1], dt.float32, tag=f"n8{h}")
                nc.scalar.activation(
                    out=n8_t[:],
                    in_=ss_t[:],
                    func=mybir.ActivationFunctionType.Sqrt,
                    scale=64.0,
                )
                rn_t = vprep.tile([128, 1], dt.float32, tag=f"rn{h}")
                nc.vector.reciprocal(out=rn_t[:], in_=n8_t[:])
                v_t = vprep.tile([128, D], dt.float32, tag=f"v{h}")
                nc.scalar.activation(
                    out=v_t[:],
                    in_=codes_t[:],
                    func=mybir.ActivationFunctionType.Copy,
                    scale=rn_t[:],
                )
                psv_t = psv.tile([128, 128], dt.float32, name=f"psv{h}", tag="psv")
                nc.tensor.transpose(out=psv_t[:], in_=v_t[:], identity=ident_f32[:])
                vT_t = vprep.tile([128, 128], dt.float32r, tag=f"vT{h}")
                nc.vector.tensor_copy(out=vT_t[:], in_=psv_t[:])
                vT.append(vT_t)

            # ---- phase B: main loop ----
            d1_strip = [constp.tile([128, 28], dt.float32, name=f"d1s{h}", tag=f"d1s{h}") for h in range(2)]
            d2_strip = [constp.tile([128, 28], dt.float32, name=f"d2s{h}", tag=f"d2s{h}") for h in range(2)]

            import contextlib

            loop_cm = tc.For_i(0, reps, 1) if reps > 1 else contextlib.nullcontext()
            with loop_cm:
              for g in range(NGROUPS):
                g0 = g * GROUP
                bank_g = bankp.tile([128, GROUP], dt.float32r, tag="bank")
                nc.sync.dma_start(out=bank_g[:], in_=bankT_d[:, g0 : g0 + GROUP])
                m_tiles = {}
                for h in range(2):
                    r0 = h * 128
                    mbg_g = maskp.tile([128, GROUP], dt.uint8, tag=f"mbg{h}")
                    nc.sync.dma_start(
                        out=mbg_g[:], in_=mbg_d[r0 : r0 + 128, g0 : g0 + GROUP]
                    )
                    mint_g = maskp.tile([128, GROUP], dt.uint8, tag=f"mint{h}")
                    nc.sync.dma_start(
                        out=mint_g[:], in_=mint_d[r0 : r0 + 128, g0 : g0 + GROUP]
                    )
                    m_tiles[h] = (mbg_g, mint_g)

                for h in range(2):
                    mbg_g, mint_g = m_tiles[h]
                    # chunk pairs: [0,1],[2,3],[4,5],[6] -> psum tiles of 1024/512
                    for pi, cis in enumerate([(0, 1), (2, 3), (4, 5), (6,)]):
                        width = CHUNK * len(cis)
                        c0 = cis[0] * CHUNK
                        kidx = g * 4 + pi  # slot in [128, 28] strip
                        psum_t = ps.tile([128, 2 * CHUNK], dt.float32, tag="psum")
                        for j, ci in enumerate(cis):
                            nc.tensor.matmul(
                                out=psum_t[:, j * CHUNK : (j + 1) * CHUNK],
                                lhsT=vT[h][:],
                                rhs=bank_g[:, ci * CHUNK : (ci + 1) * CHUNK],
                                start=True,
                                stop=False,
                            )
                        for j, ci in enumerate(cis):
                            nc.tensor.matmul(
                                out=psum_t[:, j * CHUNK : (j + 1) * CHUNK],
                                lhsT=ident_fp8[:],
                                rhs=mbg_g[:, ci * CHUNK : (ci + 1) * CHUNK].bitcast(dt.float8e5),
                                start=False,
                                stop=True,
                            )
                        e_t = ework.tile([128, 2 * CHUNK], dt.float32, tag="e")
                        nc.scalar.activation(
                            out=e_t[:, :width],
                            in_=psum_t[:, :width],
                            func=mybir.ActivationFunctionType.Exp,
                            scale=ACT_SCALE,
                            bias=bias_t[:],
                            accum_out=d1_strip[h][:, kidx : kidx + 1],
                        )
                        stt_s = scratch.tile([128, 2 * CHUNK], dt.float32, tag="stt")
                        nc.vector.scalar_tensor_tensor(
                            out=stt_s[:, :width],
                            in0=e_t[:, :width],
                            scalar=0.0,
                            in1=mint_g[:, c0 : c0 + width],
                            op0=mybir.AluOpType.add,
                            op1=mybir.AluOpType.mult,
                            accum_out=d2_strip[h][:, kidx : kidx + 1],
                        )

            # ---- phase C: finale ----
            # partials [128, 4]: cols = d1_h0, d1_h1, d2_h0, d2_h1
            parts_t = constp.tile([128, 4], dt.float32)
            for h in range(2):
                nc.vector.tensor_reduce(
                    out=parts_t[:, h : h + 1],
                    in_=d1_strip[h][:],
                    axis=mybir.AxisListType.X,
                    op=mybir.AluOpType.add,
                )
                nc.vector.tensor_reduce(
                    out=parts_t[:, 2 + h : 3 + h],
                    in_=d2_strip[h][:],
                    axis=mybir.AxisListType.X,
                    op=mybir.AluOpType.add,
                )

            cc_in = dram.tile([128, 4], dt.float32)
            cc_out = dram.tile([128, 4], dt.float32)
            nc.sync.dma_start(out=cc_in[:], in_=parts_t[:])
            nc.gpsimd.collective_compute(
                "AllReduce",
                mybir.AluOpType.add,
                replica_groups=[list(range(NCORES))],
                ins=[cc_in.opt()],
                outs=[cc_out.opt()],
            )
            sums_t = constp.tile([128, 4], dt.float32)
            nc.sync.dma_start(out=sums_t[:], in_=cc_out[:])

            ln_t = constp.tile([128, 4], dt.float32)
            nc.scalar.activation(
                out=ln_t[:], in_=sums_t[:], func=mybir.ActivationFunctionType.Ln
            )
            ldiff_t = constp.tile([128, 2], dt.float32)
            nc.vector.tensor_sub(out=ldiff_t[:], in0=ln_t[:, 0:2], in1=ln_t[:, 2:4])
            lsum_t = constp.tile([128, 1], dt.float32)
            nc.vector.tensor_reduce(
                out=lsum_t[:],
                in_=ldiff_t[:],
                axis=mybir.AxisListType.X,
                op=mybir.AluOpType.add,
            )
            # partition sum via ones-matmul: out[1,1] = sum_k lsum[k]*1
            psum_s = psv.tile([1, 1], dt.float32, tag="psum_s")
            nc.tensor.matmul(
                out=psum_s[:], lhsT=lsum_t[:], rhs=ones_t[:], start=True, stop=True
            )
            out_t = constp.tile([1, 1], dt.float32)
            nc.scalar.activation(
                out=out_t[:],
                in_=psum_s[:],
                func=mybir.ActivationFunctionType.Copy,
                scale=1.0 / B,
            )
            nc.sync.dma_start(out=out_d[:], in_=out_t[:])

    nc.compile()
    return nc


def _get_nc(reps: int = 1):
    key = ("nc", reps)
    if key not in _CACHE:
        _CACHE[key] = _build(reps)
    return _CACHE[key]


def make_in_maps(codes, bank, mask_bg, mask_int):
    codes = np.ascontiguousarray(np.asarray(codes, dtype=np.float32))
    bank = np.asarray(bank, dtype=np.float32)
    mbg_u8 = np.asarray(mask_bg).view(np.uint8) if np.asarray(mask_bg).dtype == np.bool_ else np.asarray(mask_bg).astype(np.uint8)
    mint_u8 = np.asarray(mask_int).view(np.uint8) if np.asarray(mask_int).dtype == np.bool_ else np.asarray(mask_int).astype(np.uint8)

    in_maps = []
    for c in range(NCORES):
        n0, n1 = c * S, (c + 1) * S
        bankT_c = np.zeros((D, S_PAD), dtype=np.float32)
        bankT_c[:, :S] = bank[n0:n1].T
        mbg_c = np.zeros((B, S_PAD), dtype=np.uint8)
        mbg_c[:, :S] = mbg_u8[:, n0:n1]
        mint_c = np.zeros((B, S_PAD), dtype=np.uint8)
        mint_c[:, :S] = mint_u8[:, n0:n1]
        in_maps.append(
            {
                "codes": codes,
                "bankT": np.ascontiguousarray(bankT_c),
                "mbg": mbg_c,
                "mint": mint_c,
            }
        )
    return in_maps


def kernel(codes, bank, mask_bg, mask_int):
    nc = _get_nc()
    in_maps = make_in_maps(codes, bank, mask_bg, mask_int)
    res = run_bass_kernel_spmd(nc, in_maps, core_ids=list(range(NCORES)))
    return np.float32(res.results[0]["out"][0, 0])
